# revision 7
# baseline (speedup 1.0000x reference)
"""Expert-parallel MoE "behind" block + residual on 8 Trainium2 NeuronCores.

Reference computation (fp32):
    front      = inputs[:E*C].reshape(E, C, D_IN)
    expert_out = einsum("ecd,edm->ecm", front, expert_w) + expert_b
    combined   = einsum("sec,ecm->sm", combine_weights, expert_out)
    resid      = inputs[E*C:] @ residual_w + residual_b
    out        = combined * w0[:, None] + resid * w1[:, None]

Sharding (8 cores), v2 — pipelined-AllGather layout:

  Stage 1 (expert-parallel): core e computes eo_e = front_e @ W_e [C, D_OUT]
  as c-half 0 (4 PSUM groups wide; W_e streams DRAM->SBUF once at the
  sustainable single-queue rate and stays resident) then c-quarters 2,3
  (2 groups wide, W_e read back from SBUF).  Each c-quarter of eo_e is
  AllGathered as soon as it drains -> 4 small collectives pipeline behind
  each other and complete before the combine needs them.  (The baseline's
  2 big AllGathers finished ~14/9 us after the PE went idle waiting:
  22 us + 6 us of measured stall.)

  Stage 2 (residual): (w1*resid)[S_r] @ residual_w in fp8-e4m3 DoubleRow
  (2 contraction rows/cell, half the PE cycles).  The residual path is
  ~2% of the output magnitude, so fp8 moves rel-l2 only 3.32e-3 ->
  3.42e-3 (measured on the real inputs).  The last 4 k-blocks are held
  back and run between combine quarters 2 and 3 as PE filler in case the
  last AllGather is late.

  Stage 3 (combine): accumulates (w0*cw)[S_r] @ eo_full into the same
  PSUM banks as stage 2 (exact: bias terms are zero / host-corrected).
  Contraction runs quarter-by-quarter in AllGather completion order;
  cwT's rows are host-permuted to (quarter, rank, c-within-quarter).

All device matmuls contract over the SBUF partition axis, so every DRAM
operand is laid out contraction-major on the host.

Env TRN_S3_MODE: "fp8" (default) residual fp8 DoubleRow; "bf16" fallback.
"""

import os
import numpy as np
import ml_dtypes

E, C, D_IN, D_OUT = 8, 1024, 4096, 1024
B, S = 2, 2048
TOK = B * S                 # 4096 tokens
N_CORES = 8
S_LOC = TOK // N_CORES      # 512 tokens per core
CQ = C // 4                 # c-quarter = 256
BF16 = ml_dtypes.bfloat16
FP8 = ml_dtypes.float8_e4m3fn

S3_MODE = os.environ.get("TRN_S3_MODE", "fp8")

_prog_cache = {}


def _build(s3_mode):
    import concourse.bass as bass  # noqa: F401
    import concourse.mybir as mybir
    from concourse import bacc
    from concourse.tile import TileContext, add_dep_helper

    dt = mybir.dt
    s3_fp8 = s3_mode == "fp8"
    s3_dt = dt.float8e4 if s3_fp8 else dt.bfloat16

    nc = bacc.Bacc("TRN2", target_bir_lowering=False, debug=False, num_devices=N_CORES)

    fT = nc.declare_dram_parameter("fT", [D_IN, C], dt.bfloat16, isOutput=False)
    we = nc.declare_dram_parameter("we", [D_IN, D_OUT], dt.bfloat16, isOutput=False)
    cwT = nc.declare_dram_parameter("cwT", [E * C, S_LOC], dt.bfloat16, isOutput=False)
    riT = nc.declare_dram_parameter("riT", [D_IN, S_LOC], s3_dt, isOutput=False)
    rw = nc.declare_dram_parameter("rw", [D_IN, D_OUT], s3_dt, isOutput=False)
    out = nc.declare_dram_parameter("out", [S_LOC, D_OUT], dt.float32, isOutput=True)

    # variant tag so differently-compiled builds never share a jax cache entry
    nc.dram_tensor(f"variant_v2_{s3_mode}", [1, 1], dt.float32)

    ag_in = [nc.dram_tensor(f"ag_in{q}", [CQ, D_OUT], dt.bfloat16) for q in range(4)]
    ag_out = [nc.dram_tensor(f"ag_out{q}", [N_CORES * CQ, D_OUT], dt.bfloat16,
                             addr_space="Shared") for q in range(4)]

    KT = D_IN // 128            # 32 contraction tiles
    NFREE = 512
    NJ = D_OUT // NFREE         # 2
    rearr = lambda a: a.rearrange("(n p) d -> p n d", p=128)

    eoag_bufs = 3 if s3_fp8 else 2
    cw_bufs = 4 if s3_fp8 else 3
    with TileContext(nc) as tc:
        with tc.tile_pool(name="p_ft", bufs=4) as p_ft, \
             tc.tile_pool(name="p_sb", bufs=1) as p_sb, \
             tc.tile_pool(name="p_cw", bufs=cw_bufs) as p_cw, \
             tc.tile_pool(name="p_eoag", bufs=eoag_bufs) as p_eoag, \
             tc.tile_pool(name="p_rw", bufs=3) as p_rw, \
             tc.tile_pool(name="psum", bufs=1, space="PSUM") as p_ps:

            # ---- persistent SBUF tensors (per-partition KiB in comments) ---
            we_sb = p_sb.tile([128, KT, D_OUT], dt.bfloat16)          # 64K
            eo_sb = p_sb.tile([128, C // 128, D_OUT], dt.bfloat16)    # 16K
            ri_sb = p_sb.tile([128, KT, S_LOC], s3_dt)                # 16/32K
            out_sb = p_sb.tile([128, S_LOC // 128, D_OUT], dt.float32)  # 16K
            rw_sb = (p_sb.tile([128, KT, D_OUT], s3_dt, name="rw_sb")
                     if s3_fp8 else None)                             # 32K
            # (residual operand loads are emitted after the we stream below)

            def ps_tile(i):
                return p_ps.tile([128, D_OUT], dt.float32, name=f"ps{i}", tag=f"ps_{i}")

            def trigger_ag(q):
                nc.gpsimd.dma_start(out=rearr(ag_in[q][:]),
                                    in_=eo_sb[:, 2 * q:2 * q + 2, :])
                return nc.gpsimd.collective_compute(
                    "AllGather", mybir.AluOpType.bypass,
                    replica_groups=[list(range(N_CORES))],
                    ins=[ag_in[q][:].opt()], outs=[ag_out[q][:].opt()])

            # ---- Stage 1a: eo c-half 0 (4 groups wide) ---------------------
            # Queue plan: the two HWDGE rings (sync/scalar) share a ~9-deep
            # DMA-completion semaphore pool, each event held until its
            # consumer resets it -- so prefetch runway is ~9 outstanding
            # transfers TOTAL across both rings.  The SWDGE queue has its own
            # pool and ~190 GB/s.  So: sync = ft only (few, large blocks);
            # scalar = eoag only (collective-gated, pinned late);
            # gpsimd/SWDGE = all bulk streams (we, ri, rw, later cw).
            psums = [ps_tile(i) for i in range(4)]
            # (A HAM pre-warm burst of dummy matmuls here was measured NET
            # NEGATIVE: the static burst ends before the first operand DMAs
            # land, and the idle gap in between re-throttles the clock.)
            last_we_dma = None
            for blk in range(KT // 4):
                k0 = blk * 512
                ft_t = p_ft.tile([128, 4, 2 * CQ], dt.bfloat16, tag="ft",
                                 name=f"ftA_{blk}")
                if blk == 0:
                    # fine-grained first block: first matmul starts after
                    # 128 KiB of ft and 128 KiB of we
                    for sub in range(4):
                        nc.sync.dma_start(
                            out=ft_t[:, sub:sub + 1, :],
                            in_=rearr(fT[k0 + 128 * sub:k0 + 128 * (sub + 1), 0:2 * CQ]))
                        if sub == 0:
                            for jh in range(2):
                                last_we_dma = nc.gpsimd.dma_start(
                                    out=we_sb[:, 0:1, jh * NFREE:(jh + 1) * NFREE],
                                    in_=rearr(we[k0:k0 + 128,
                                                 jh * NFREE:(jh + 1) * NFREE]))
                        else:
                            last_we_dma = nc.gpsimd.dma_start(
                                out=we_sb[:, sub:sub + 1, :],
                                in_=rearr(we[k0 + 128 * sub:k0 + 128 * (sub + 1), :]))
                else:
                    nc.sync.dma_start(out=ft_t, in_=rearr(fT[k0:k0 + 512, 0:2 * CQ]))
                    # per-kt we blocks: each completion event gates only 8
                    # matmuls instead of 16 -> finer pipelining (same lesson
                    # as the combine: granularity wins with >=2x margin)
                    for h in range(4):
                        last_we_dma = nc.gpsimd.dma_start(
                            out=we_sb[:, 4 * blk + h:4 * blk + h + 1, :],
                            in_=rearr(we[k0 + 128 * h:k0 + 128 * (h + 1), :]))
                for sub in range(4):
                    kt = blk * 4 + sub
                    for i in range(4):
                        for j in range(NJ):
                            nc.tensor.matmul(
                                psums[i][:, j * NFREE:(j + 1) * NFREE],
                                ft_t[:, sub, i * 128:(i + 1) * 128],
                                we_sb[:, kt, j * NFREE:(j + 1) * NFREE],
                                start=(kt == 0), stop=(kt == KT - 1))
            # residual operands on SWDGE behind the we stream (pinned so the
            # scheduler can't hoist them into the middle of it)
            ri_dma = nc.gpsimd.dma_start(out=ri_sb, in_=rearr(riT[:, :]))
            add_dep_helper(ri_dma.ins, last_we_dma.ins, False, "ri after we stream")
            rw_dma = ri_dma
            if s3_fp8:
                rw_dma = nc.gpsimd.dma_start(out=rw_sb, in_=rearr(rw[:, :]))
                add_dep_helper(rw_dma.ins, ri_dma.ins, False, "rw after ri")

            for q in range(2):           # drain + ship quarters 0,1
                for g in range(2):
                    i = q * 2 + g
                    for j in range(NJ):
                        nc.vector.tensor_copy(
                            out=eo_sb[:, i, j * NFREE:(j + 1) * NFREE],
                            in_=psums[i][:, j * NFREE:(j + 1) * NFREE])
                trigger_ag(q)

            # ---- Stage 1b/1c: c-quarters 2,3 (2 groups wide, we from SBUF) -
            # ft alternates between both HWDGE rings (scalar is otherwise
            # idle here): doubles prefetch runway against AG bandwidth
            # contention, which stalled this phase up to 19us in slow runs
            last_ft_dma = None
            last_trig = None
            for q in (2, 3):
                c0 = q * CQ
                psq = [ps_tile(2 * (q - 2)), ps_tile(2 * (q - 2) + 1)]
                for blk in range(KT // 4):
                    k0 = blk * 512
                    ft_t = p_ft.tile([128, 4, CQ], dt.bfloat16, tag="ft",
                                     name=f"ftQ{q}_{blk}")
                    eng = nc.sync if blk % 2 == 0 else nc.scalar
                    last_ft_dma = eng.dma_start(
                        out=ft_t, in_=rearr(fT[k0:k0 + 512, c0:c0 + CQ]))
                    for sub in range(4):
                        kt = blk * 4 + sub
                        for g in range(2):
                            for j in range(NJ):
                                nc.tensor.matmul(
                                    psq[g][:, j * NFREE:(j + 1) * NFREE],
                                    ft_t[:, sub, g * 128:(g + 1) * 128],
                                    we_sb[:, kt, j * NFREE:(j + 1) * NFREE],
                                    start=(kt == 0), stop=(kt == KT - 1))
                for g in range(2):
                    for j in range(NJ):
                        nc.vector.tensor_copy(
                            out=eo_sb[:, 2 * q + g, j * NFREE:(j + 1) * NFREE],
                            in_=psq[g][:, j * NFREE:(j + 1) * NFREE])
                last_trig = trigger_ag(q)

            # ---- Stage 2: residual partial (w1 folded host-side) -----------
            psums = [ps_tile(i) for i in range(4)]

            def resid_blocks_fp8(b0, b1):
                for kb in range(b0, b1):
                    for i in range(4):
                        for j in range(4):
                            nc.tensor.matmul(
                                psums[i][:, j * 256:(j + 1) * 256],
                                ri_sb[:, 2 * kb:2 * kb + 2, i * 128:(i + 1) * 128],
                                rw_sb[:, 2 * kb:2 * kb + 2, j * 256:(j + 1) * 256],
                                start=(kb == 0), stop=False,
                                perf_mode=mybir.MatmulPerfMode.DoubleRow)

            def resid_blocks_bf16(b0, b1):
                for blk in range(b0, b1):
                    rw_t = p_rw.tile([128, 4, D_OUT], dt.bfloat16, tag="rw",
                                     name=f"rw_{blk}")
                    nc.gpsimd.dma_start(out=rw_t,
                                        in_=rearr(rw[blk * 512:(blk + 1) * 512, :]))
                    for sub in range(4):
                        kt = blk * 4 + sub
                        for i in range(4):
                            for j in range(NJ):
                                nc.tensor.matmul(
                                    psums[i][:, j * NFREE:(j + 1) * NFREE],
                                    ri_sb[:, kt, i * 128:(i + 1) * 128],
                                    rw_t[:, sub, j * NFREE:(j + 1) * NFREE],
                                    start=(kt == 0), stop=False)

            # All residual blocks run here: AG3 finishes ~252us vs combine
            # q3's need at ~286us, and late-AG draws come with late stage-1
            # (which pushes the need out too), so no filler insurance needed
            if s3_fp8:
                resid_blocks_fp8(0, 16)
            else:
                resid_blocks_bf16(0, 8)

            # ---- Stage 3: combine, quarter-by-quarter ----------------------
            # The eoag loads wait on the AllGathers, and Tile multiplexes DMA
            # completion events onto a small shared semaphore pool across all
            # rings: a collective-gated eoag event enqueued before a stage-1
            # operand load's event stalls the PE on that operand until the
            # collective finishes (measured 16us).  So pin every eoag load
            # after the whole stage-1 stream, in block order; same for cw on
            # the SWDGE queue behind the last collective trigger.
            # eoag alternates between both HWDGE rings (each otherwise idle
            # during the combine): halves per-ring load from ~113 GB/s (the
            # measured per-ring edge) to a comfortable ~57 GB/s
            prev_eoag = {0: None, 1: None}
            prev_cw = None

            def combine_block(q, blk, finish=False):
                nonlocal prev_cw
                r0 = blk * 512
                cw_t = p_cw.tile([128, 4, S_LOC], dt.bfloat16, tag="cw",
                                 name=f"cw_{q}_{blk}")
                cw_dma = nc.gpsimd.dma_start(
                    out=cw_t, in_=rearr(cwT[q * 2048 + r0:q * 2048 + r0 + 512, :]))
                add_dep_helper(cw_dma.ins, (prev_cw or last_trig).ins, False,
                               "cw after AG triggers, in block order")
                prev_cw = cw_dma
                eo_t = p_eoag.tile([128, 4, D_OUT], dt.bfloat16, tag="eoag",
                                   name=f"eoag_{q}_{blk}")
                ring = (q * 4 + blk) % 2
                eng = nc.scalar if ring == 0 else nc.sync
                eoag_dma = eng.dma_start(out=eo_t,
                                         in_=rearr(ag_out[q][r0:r0 + 512, :]))
                for dep in ([prev_eoag[ring]] if prev_eoag[ring] is not None
                            else [last_ft_dma, rw_dma]):
                    add_dep_helper(eoag_dma.ins, dep.ins, False,
                                   "eoag after stage-1 loads, in block order")
                prev_eoag[ring] = eoag_dma
                if not finish:
                    for sub in range(4):
                        for i in range(4):
                            for j in range(NJ):
                                nc.tensor.matmul(
                                    psums[i][:, j * NFREE:(j + 1) * NFREE],
                                    cw_t[:, sub, i * 128:(i + 1) * 128],
                                    eo_t[:, sub, j * NFREE:(j + 1) * NFREE],
                                    start=False, stop=False)
                else:
                    # last block: finish PSUM groups one at a time so the
                    # drains and output DMAs overlap the remaining matmuls;
                    # all of group i's matmuls are issued before its copies
                    # so a copy never sits between two matmul runs
                    for i in range(4):
                        for j in range(NJ):
                            for sub in range(4):
                                nc.tensor.matmul(
                                    psums[i][:, j * NFREE:(j + 1) * NFREE],
                                    cw_t[:, sub, i * 128:(i + 1) * 128],
                                    eo_t[:, sub, j * NFREE:(j + 1) * NFREE],
                                    start=False, stop=(sub == 3))
                        for j in range(NJ):
                            nc.vector.tensor_copy(
                                out=out_sb[:, i, j * NFREE:(j + 1) * NFREE],
                                in_=psums[i][:, j * NFREE:(j + 1) * NFREE])
                            nc.sync.dma_start(
                                out=rearr(out[i * 128:(i + 1) * 128,
                                              j * NFREE:(j + 1) * NFREE]),
                                in_=out_sb[:, i:i + 1, j * NFREE:(j + 1) * NFREE])

            for q in range(4):
                for blk in range(4):
                    combine_block(q, blk, finish=(q == 3 and blk == 3))

    nc.finalize()
    return nc


def _get_prog(s3_mode):
    if s3_mode not in _prog_cache:
        _prog_cache[s3_mode] = _build(s3_mode)
    return _prog_cache[s3_mode]


def _prep_in_maps(inputs, expert_w, residual_w, combine_weights, residual_weight,
                  s3_mode):
    s3_np = FP8 if s3_mode == "fp8" else BF16
    front = inputs[:E * C].reshape(E, C, D_IN)
    resid = inputs[E * C:]                       # [TOK, D_IN]
    rwt = residual_weight.reshape(TOK, 2)
    w0, w1 = rwt[:, 0], rwt[:, 1]

    rw_cast = np.ascontiguousarray(residual_w.astype(s3_np))
    resid_s = resid * w1[:, None]                # fold w1 (fp32)
    in_maps = []
    for r in range(N_CORES):
        sl = slice(r * S_LOC, (r + 1) * S_LOC)
        fT = np.ascontiguousarray(front[r].T.astype(BF16))               # [D_IN, C]
        we = np.ascontiguousarray(expert_w[r].astype(BF16))              # [D_IN, D_OUT]
        cw_s = combine_weights[sl] * w0[sl, None, None]                  # [S_LOC, E, C]
        # contraction rows ordered (c-quarter, rank, c-within-quarter) to
        # match the chunked AllGather concatenation
        cwT = np.ascontiguousarray(
            cw_s.reshape(S_LOC, E, 4, CQ).transpose(2, 1, 3, 0).reshape(E * C, S_LOC)
            .astype(BF16))
        riT = np.ascontiguousarray(resid_s[sl].T.astype(s3_np))          # [D_IN, S_LOC]
        in_maps.append({"fT": fT, "we": we, "cwT": cwT, "riT": riT, "rw": rw_cast})
    return in_maps


def _run(inputs, expert_w, expert_b, residual_w, residual_b,
         combine_weights, residual_weight, s3_mode=None, trace=False):
    import jax
    try:
        if jax.config.jax_compilation_cache_dir is None:
            jax.config.update("jax_compilation_cache_dir", "/tmp/jax_cache_trn_moe")
            jax.config.update("jax_persistent_cache_min_compile_time_secs", 0.5)
    except Exception:
        pass
    from concourse.bass_utils import run_bass_kernel_spmd

    s3_mode = s3_mode or S3_MODE
    inputs = np.asarray(inputs, dtype=np.float32)
    expert_w = np.asarray(expert_w, dtype=np.float32)
    expert_b = np.asarray(expert_b, dtype=np.float32)
    residual_w = np.asarray(residual_w, dtype=np.float32)
    residual_b = np.asarray(residual_b, dtype=np.float32)
    combine_weights = np.asarray(combine_weights, dtype=np.float32)
    residual_weight = np.asarray(residual_weight, dtype=np.float32)

    nc = _get_prog(s3_mode)
    in_maps = _prep_in_maps(inputs, expert_w, residual_w, combine_weights,
                            residual_weight, s3_mode)
    res = run_bass_kernel_spmd(nc, in_maps, list(range(N_CORES)), trace=trace)
    out = np.concatenate([res.results[r]["out"] for r in range(N_CORES)], axis=0)

    # exact bias contributions (zero in practice, but keep the math honest)
    rwt = residual_weight.reshape(TOK, 2)
    if residual_b.any():
        out = out + rwt[:, 1:2] * residual_b[None, :]
    if expert_b.any():
        cs = combine_weights.sum(axis=2)                    # [TOK, E]
        out = out + rwt[:, 0:1] * (cs @ expert_b)
    return out.reshape(B, S, D_OUT).astype(np.float32), res


def kernel(**kw):
    out, _ = _run(**kw)
    return out
